# revision 41
# baseline (speedup 1.0000x reference)
"""Llama GQA attention prefill (B=1, Q=1024, PAST=3072) on 8 TRN2 NeuronCores.

Sharding: tensor-parallel by head. Core g owns KV head g and its 4 query
heads (GQA group), row-shard of Wo; bf16 partial outputs summed on host.

Inputs are host-cast to bf16; the exp tiles and V stay f32r (exp on ACT
is ~20% faster writing f32r, and exact e improves accuracy); PSUM
accumulation is fp32. Key hardware constraint honored throughout: a
matmul with start_tensor_calc=True zeroes its ENTIRE 2KB PSUM bank, so
every bank has exactly one start (first group emitted) and one stop.

Per-core pipeline:
  1. Projections with resident weights, seq in 4 quarters of 256, xt
     streamed per quarter. Q^T and K^T are produced directly by using
     the weight chunk as the stationary operand (no PE transposes); V is
     produced natural. RoPE runs on DVE over partition halves (d and
     d+64 pair up across the partition dim in the ^T layouts). Weight /
     xt / table DMAs are spread over the scalar-HWDGE, SWDGE and
     SP-HWDGE queues so delivery stays ahead of the PE.
  2. Attention per head in scores^T orientation [kv, q], software-
     pipelined two iterations deep: scores(i) stream on PE, one unsplit
     exp(i) on ACT, then AV(i-2) — so the PE and ACT both stay ~90%
     busy. Denominators use e-tiles pair-summed on DVE and ones-column
     matmuls deferred one further pair (4x fewer den matmul columns).
  3. Softmax normalization: den accumulated in one PSUM bank at
     partition offsets 0/32, reciprocal_approx_fast, broadcast matmul,
     deferred into the next head's PE stream.
  4. Output projection in [128,1024] PSUM super-tiles with a 3-deep
     rotation (2 psA slots + the freed o_ps slot); the two halves
     evacuate on DVE and ACT in parallel; bf16 stores alternate between
     the two HWDGE queues.
"""

import sys

sys.path.insert(0, "/opt/trn_rl_repo")

import math

import numpy as np

B, Q, PAST = 1, 1024, 3072
KV = PAST + Q
HID, NH, NKV, HD = 4096, 32, 8, 128
GROUPS = NH // NKV
THETA = 10000.0
N_CORES = 8
H_PER_CORE = NH // N_CORES  # 4 query heads per core
DH = H_PER_CORE * HD        # 512 contraction dims per core in Wo
P = 128
HC = HID // P               # 32 hidden chunks
KT = KV // P                # 32 kv tiles
QT = Q // 512               # 2 q tiles of 512
NQTR = 4                    # seq quarters
QTR = Q // NQTR             # 256
SCALE = 1.0 / math.sqrt(HD)

_cache = {}


def _build(mask_nonzero: bool, debug: bool = False):
    import concourse.bacc as bacc
    import concourse.mybir as mybir
    import concourse.tile as tile

    f32 = mybir.dt.float32
    f32r = mybir.dt.float32r
    bf16 = mybir.dt.bfloat16
    AF = mybir.ActivationFunctionType
    OP = mybir.AluOpType

    nc = bacc.Bacc("TRN2", target_bir_lowering=False, num_swdge_queues=4)

    # ---- DRAM tensors (per-core shards, host-prepared layouts) ----
    xt_d = nc.dram_tensor("xt", [HID, Q], bf16, kind="ExternalInput")          # hidden^T
    wq_d = nc.dram_tensor("wqt", [HID, DH], bf16, kind="ExternalInput")        # Wq_shard^T
    wkv_d = nc.dram_tensor("wkvt", [HID, 2 * HD], bf16, kind="ExternalInput")  # [Wk|Wv]_shard^T
    wo_d = nc.dram_tensor("wot", [DH, HID], bf16, kind="ExternalInput")        # Wo_shard^T
    pkt_d = nc.dram_tensor("past_kt", [HD, PAST], bf16, kind="ExternalInput")  # past_k^T
    pv_d = nc.dram_tensor("past_v", [PAST, HD], f32, kind="ExternalInput")     # natural
    cos_d = nc.dram_tensor("cos_t", [P, Q], f32, kind="ExternalInput")         # cos, ^T layout
    sin_d = nc.dram_tensor("sinS_t", [P, Q], f32, kind="ExternalInput")        # +-sin, ^T layout
    if mask_nonzero:
        emask_d = nc.dram_tensor("expmask_t", [KV, Q], bf16, kind="ExternalInput")
    out_d = nc.dram_tensor("out_partial", [Q, HID], bf16, kind="ExternalOutput")
    if debug:
        dbg_qt_d = nc.dram_tensor("dbg_qt", [P, H_PER_CORE * Q], bf16, kind="ExternalOutput")
        dbg_kt_d = nc.dram_tensor("dbg_kt", [P, KV], bf16, kind="ExternalOutput")
        dbg_v_d = nc.dram_tensor("dbg_v", [P, KT * HD], f32, kind="ExternalOutput")
        dbg_den_d = nc.dram_tensor("dbg_den", [H_PER_CORE, Q], f32, kind="ExternalOutput")
        dbg_o_d = nc.dram_tensor("dbg_o", [P, H_PER_CORE * Q], bf16, kind="ExternalOutput")
        dbg_on_d = nc.dram_tensor("dbg_onorm", [P, H_PER_CORE * Q], bf16, kind="ExternalOutput")
        dbg_wq_d = nc.dram_tensor("dbg_wq", [P, HC * DH], bf16, kind="ExternalOutput")
        dbg_wkv_d = nc.dram_tensor("dbg_wkv", [P, HC * 2 * HD], bf16, kind="ExternalOutput")

    with tile.TileContext(nc) as tc, \
         nc.allow_low_precision(reason="bf16 matmul pipeline; softmax stats stay fp32"):
        with tc.tile_pool(name="const", bufs=1) as const_pool, \
             tc.tile_pool(name="xstream", bufs=2) as xtp, \
             tc.tile_pool(name="work", bufs=2) as work, \
             tc.tile_pool(name="estream", bufs=4) as epool, \
             tc.tile_pool(name="wosb", bufs=3) as wosb, \
             tc.tile_pool(name="psA", bufs=2, space="PSUM") as psA, \
             tc.tile_pool(name="psO", bufs=1, space="PSUM") as psO, \
             tc.tile_pool(name="psC", bufs=2, space="PSUM") as psC:

            # ---- persistent SBUF tensors ----
            wq_sb = const_pool.tile([P, HC, DH], bf16)         # Wq^T chunks
            wkv_sb = const_pool.tile([P, HC, 2 * HD], bf16)    # [Wk|Wv]^T chunks
            wo_sb = const_pool.tile([P, H_PER_CORE, HID // 512, 512], bf16)
            kt_sb = const_pool.tile([P, KV], bf16)             # K^T (past + new)
            v_sb = const_pool.tile([P, KT, HD], f32r)          # V natural
            qt_sb = const_pool.tile([P, H_PER_CORE, Q], bf16)  # Q^T per head
            o_sb = const_pool.tile([P, H_PER_CORE, Q], bf16)   # O^T per head
            cos_sb = const_pool.tile([P, Q], f32)
            sin_sb = const_pool.tile([P, Q], f32)              # signed sin
            ones_col = const_pool.tile([P, 1], bf16)
            nc.vector.memset(ones_col[:], 1.0)
            ones_row = const_pool.tile([1, P], f32)
            nc.vector.memset(ones_row[:], 1.0)

            # ---- prologue DMAs, interleaved so hc=0 arrives first ----
            # Wq rides the SP HWDGE queue; xt quarter 0 + Wkv ride the
            # Pool SWDGE queue — two parallel streams, both ahead of the
            # PE's per-chunk consumption rate.
            # scalar HWDGE issues fast (~0.7us) -> Wq there; the sync
            # queue issues strided DMAs slowly (~3.6us) -> only the two
            # contiguous cos/sin tables; xt + Wkv interleave on SWDGE
            xq0 = xtp.tile([P, HC, QTR], bf16, tag="xt", name="xq0")
            # Deadline-scheduled prologue (measured issue costs: scalar
            # ~0.7us, SWDGE ~1.1us, sync ~3.6us per strided DMA):
            #  - Wkv first blocks ride SWDGE ahead of xt so the very first
            #    matmul isn't gated by the slow sync queue;
            #  - cos/sin go first on sync (needed only at the first RoPE
            #    drain ~37us);
            #  - Wkv's three late blocks stay on sync, where even 3.6us
            #    issues beat their consumption deadlines.
            def wkv_dma(eng, c0, c1):
                eng.dma_start(
                    wkv_sb[:, c0:c1, :],
                    wkv_d[c0 * P : c1 * P, :].rearrange("(c p) d -> p c d", p=P),
                )
            wkv_dma(nc.gpsimd, 0, 2)
            wkv_dma(nc.gpsimd, 2, 8)
            nc.sync.dma_start(cos_sb[:], cos_d[:])
            nc.sync.dma_start(sin_sb[:], sin_d[:])
            wkv_dma(nc.sync, 8, 16)
            wkv_dma(nc.sync, 16, 24)
            wkv_dma(nc.sync, 24, 32)
            blocks = [(0, 2), (2, 8), (8, 16), (16, 24), (24, 32)]
            for c0, c1 in blocks:
                nc.scalar.dma_start(
                    wq_sb[:, c0:c1, :],
                    wq_d[c0 * P : c1 * P, :].rearrange("(c p) d -> p c d", p=P),
                )
                nc.gpsimd.dma_start(
                    xq0[:, c0:c1, :],
                    xt_d[c0 * P : c1 * P, 0:QTR].rearrange("(c p) q -> p c q", p=P),
                )

            # ---- phase 1: projections + RoPE, per seq quarter ----
            # later quarters stream in 8-chunk sub-DMAs so consumption can
            # begin before the whole quarter lands
            xqs = [xq0]
            for qtr in range(1, NQTR):
                t = xtp.tile([P, HC, QTR], bf16, tag="xt", name=f"xq{qtr}")
                for c0 in range(0, HC, 16):
                    nc.gpsimd.dma_start(
                        t[:, c0 : c0 + 16, :],
                        xt_d[c0 * P : (c0 + 16) * P, qtr * QTR : (qtr + 1) * QTR]
                        .rearrange("(c p) q -> p c q", p=P),
                    )
                xqs.append(t)
            # past KV + Wo after the xt stream is queued
            nc.gpsimd.dma_start(kt_sb[:, 0:PAST], pkt_d[:])
            nc.gpsimd.dma_start(
                v_sb[:, 0 : PAST // P, :], pv_d.rearrange("(t p) d -> p t d", p=P)
            )
            nc.gpsimd.dma_start(
                wo_sb[:], wo_d.rearrange("(h p) (n c) -> p h n c", p=P, c=512)
            )

            for qtr in range(NQTR):
                xq = xqs[qtr]
                q_ps = psA.tile([P, H_PER_CORE, QTR], f32, tag="A", name=f"qp{qtr}")
                kv_ps = psC.tile([P, 2 * QTR], f32, tag="C", name=f"kvp{qtr}")
                for hc in range(HC):
                    st = hc == 0
                    sp = hc == HC - 1
                    x_sl = xq[:, hc, :]
                    # PSUM start_tensor_calc zeroes the WHOLE 2KB bank, so
                    # each bank gets exactly one start (first group emitted)
                    # and one stop (last group); groups in between accumulate
                    # onto the bank wiped by the first group's start.
                    # bank0: h0,h1 | bank1: h2,h3 | kv bank: K,V0,V1.
                    # The two 128-col V matmuls interleave between 256-col
                    # ones so their ldweights stay hidden.
                    nc.tensor.matmul(q_ps[:, 0, :], wq_sb[:, hc, 0:P], x_sl,
                                     start=st, stop=False)
                    nc.tensor.matmul(q_ps[:, 1, :], wq_sb[:, hc, P : 2 * P], x_sl,
                                     start=False, stop=sp)
                    nc.tensor.matmul(kv_ps[:, 0:QTR], wkv_sb[:, hc, 0:HD], x_sl,
                                     start=st, stop=False)
                    nc.tensor.matmul(q_ps[:, 2, :], wq_sb[:, hc, 2 * P : 3 * P], x_sl,
                                     start=st, stop=False)
                    nc.tensor.matmul(kv_ps[:, QTR : QTR + P], x_sl[:, 0:P],
                                     wkv_sb[:, hc, HD : 2 * HD], start=False, stop=False)
                    nc.tensor.matmul(q_ps[:, 3, :], wq_sb[:, hc, 3 * P : 4 * P], x_sl,
                                     start=False, stop=sp)
                    nc.tensor.matmul(kv_ps[:, QTR + P : 2 * QTR], x_sl[:, P : 2 * P],
                                     wkv_sb[:, hc, HD : 2 * HD], start=False, stop=sp)
                # drain: RoPE Q/K on DVE (partition-half rotate), V evac
                sl = slice(qtr * QTR, (qtr + 1) * QTR)
                cos_q = cos_sb[:, sl]
                sin_q = sin_sb[:, sl]
                for h in range(H_PER_CORE):
                    src = q_ps[:, h, :]
                    tmp = work.tile([P, QTR], f32, tag="ropeT", name=f"t{qtr}_{h}", bufs=1)
                    qc = work.tile([P, QTR], f32, tag="ropeC", name=f"c{qtr}_{h}", bufs=1)
                    nc.vector.tensor_tensor(tmp[0:64, :], src[64:P, :],
                                            sin_q[0:64, :], OP.mult)
                    nc.vector.tensor_tensor(tmp[64:P, :], src[0:64, :],
                                            sin_q[64:P, :], OP.mult)
                    nc.vector.tensor_tensor(qc[:], src, cos_q, OP.mult)
                    nc.vector.tensor_tensor(qt_sb[:, h, sl], qc[:], tmp[:], OP.add)
                src = kv_ps[:, 0:QTR]
                tmp = work.tile([P, QTR], f32, tag="ropeT", name=f"tk{qtr}", bufs=1)
                qc = work.tile([P, QTR], f32, tag="ropeC", name=f"ck{qtr}", bufs=1)
                nc.vector.tensor_tensor(tmp[0:64, :], src[64:P, :],
                                        sin_q[0:64, :], OP.mult)
                nc.vector.tensor_tensor(tmp[64:P, :], src[0:64, :],
                                        sin_q[64:P, :], OP.mult)
                nc.vector.tensor_tensor(qc[:], src, cos_q, OP.mult)
                nc.vector.tensor_tensor(kt_sb[:, PAST + qtr * QTR : PAST + (qtr + 1) * QTR],
                                        qc[:], tmp[:], OP.add)
                for i in range(2):
                    nc.vector.tensor_copy(
                        v_sb[:, PAST // P + 2 * qtr + i, :],
                        kv_ps[:, QTR + i * P : QTR + (i + 1) * P],
                    )

            # ---- phase 2: attention, software-pipelined over (h, kt) ----
            # Per iteration: scores(i) on PE, one unsplit exp(i) on ACT,
            # then the PE work of iteration i-2 (AV + quad-den), so the PE
            # never waits on ACT and ACT streams exps back-to-back.
            # Denominators use e-tiles quad-summed on DVE (4x fewer
            # ones-column matmuls).
            from collections import deque

            pending = deque()   # (h, kt, e_t, esum, o_ps, den_ps)
            norm_fin = [None]

            pending_den = deque()  # (kt, esum, den_ps) — extra-deferred

            def emit_den(kt, esum, den_ps):
                # den qt0/qt1 share one PSUM bank (rows 0 and 32): the
                # first quad's qt0 owns the bank start, last quad's qt1
                # the stop.
                for qt in range(QT):
                    nc.tensor.matmul(
                        den_ps[qt * 32 : qt * 32 + 1, :], ones_col[:],
                        esum[:, qt * 512 : (qt + 1) * 512],
                        start=(kt == 1 and qt == 0),
                        stop=(kt == KT - 1 and qt == QT - 1),
                    )

            def emit_deferred(h, kt, e_t, esum, o_ps, den_ps):
                st = kt == 0
                sp = kt == KT - 1
                for qt in range(QT):
                    nc.tensor.matmul(
                        o_ps[:, qt * 512 : (qt + 1) * 512], v_sb[:, kt, :],
                        e_t[:, qt * 512 : (qt + 1) * 512],
                        start=st, stop=sp,
                    )
                if kt % 2 == 1:
                    # extra deferral (one pair) so the GpSimd esum adds are
                    # never on the PE's critical path
                    pending_den.append((kt, esum, den_ps))
                    if not sp:
                        while len(pending_den) > 1:
                            emit_den(*pending_den.popleft())
                if sp:
                    while pending_den:
                        emit_den(*pending_den.popleft())
                    emit_norm(h, o_ps, den_ps)

            def emit_norm(h, o_ps, den_ps):
                # evacuate O^T raw; start 1/den on DVE; defer the
                # PE-visible broadcast into the next head's stream
                nc.vector.tensor_copy(o_sb[:, h, :], o_ps[:])
                den_sb = work.tile([1, Q], f32, tag="densb", name=f"dn{h}")
                for qt in range(QT):
                    nc.vector.tensor_copy(
                        den_sb[:, qt * 512 : (qt + 1) * 512],
                        den_ps[qt * 32 : qt * 32 + 1, :],
                    )
                recip = work.tile([1, Q], f32, tag="recip", name=f"rc{h}")
                nc.vector.reciprocal_approx_fast(recip[:], den_sb[:])
                if debug:
                    nc.sync.dma_start(dbg_den_d[h : h + 1, :], den_sb[:])
                    nc.sync.dma_start(
                        dbg_o_d[:, h * Q : (h + 1) * Q], o_sb[:, h, :]
                    )

                def _finalize(h=h, recip=recip):
                    bc_ps = psC.tile([P, 512], f32, tag="C", name=f"bc{h}")
                    bc_sb = work.tile([P, 512], f32, tag="bcast", name=f"bcs{h}", bufs=1)
                    for qt in range(QT):
                        sl = slice(qt * 512, (qt + 1) * 512)
                        nc.tensor.matmul(bc_ps[:], ones_row[:], recip[:, sl])
                        nc.vector.tensor_copy(bc_sb[:], bc_ps[:])
                        nc.vector.tensor_tensor(
                            o_sb[:, h, sl], o_sb[:, h, sl], bc_sb[:], OP.mult
                        )

                norm_fin[0] = _finalize

            o_ps = den_ps = esum = e_prev = None
            for h in range(H_PER_CORE):
                o_ps = psO.tile([P, Q], f32, tag="O", name=f"o{h}")
                den_ps = psC.tile([P, 512], f32, tag="C", name=f"den{h}")
                for kt in range(KT):
                    s_ps = psA.tile([P, Q], f32, tag="A", name=f"s{h}_{kt}")
                    for qt in range(QT):
                        nc.tensor.matmul(
                            s_ps[:, qt * 512 : (qt + 1) * 512],
                            kt_sb[:, kt * P : (kt + 1) * P],
                            qt_sb[:, h, qt * 512 : (qt + 1) * 512],
                        )
                    e_t = epool.tile([P, Q], f32r, tag="E", name=f"e{h}_{kt}")
                    nc.scalar.activation(e_t[:], s_ps[:], AF.Exp, scale=SCALE)
                    if mask_nonzero:
                        em_t = epool.tile([P, Q], bf16, tag="em", name=f"em{h}_{kt}")
                        nc.gpsimd.dma_start(em_t[:], emask_d[kt * P : (kt + 1) * P, :])
                        nc.vector.tensor_tensor(e_t[:], e_t[:], em_t[:], OP.mult)
                    if kt % 2 == 0:
                        e_prev = e_t
                    else:
                        esum = epool.tile([P, Q], bf16, tag="ES",
                                          name=f"es{h}_{kt // 2}", bufs=2)
                        # pair-sum on DVE (~1.5us); den matmuls are
                        # deferred a further pair so the add latency is
                        # never on the PE's critical path
                        nc.vector.tensor_tensor(esum[:], e_prev[:], e_t[:], OP.add)
                    pending.append((h, kt, e_t, esum, o_ps, den_ps))
                    while len(pending) > 2:
                        emit_deferred(*pending.popleft())
                    if kt == 14 and norm_fin[0] is not None:
                        norm_fin[0]()
                        norm_fin[0] = None
            while pending:
                emit_deferred(*pending.popleft())
            norm_fin[0]()
            norm_fin[0] = None

            # ---- phase 3: output projection (partial, summed on host) ----
            # [128, 1024] super-tiles (two 512-wide n-blocks, one PSUM bank
            # each) from the psA pool — a deeper effective rotation than the
            # single psC slot pair, so evacuation latency never stalls the PE
            for qc in range(Q // P):
                for np2 in range(HID // 1024):
                    idx = qc * (HID // 1024) + np2
                    # 3-deep effective rotation: two psA slots + the freed
                    # attention o_ps slot
                    pool = psA if idx % 3 < 2 else psO
                    tag = "A" if idx % 3 < 2 else "O"
                    w_ps = pool.tile([P, Q], f32, tag=tag, name=f"wps{idx}")
                    # h=3 last so its normalization has maximal slack
                    for nn in range(2):
                        for h in range(H_PER_CORE):
                            nc.tensor.matmul(
                                w_ps[:, nn * 512 : (nn + 1) * 512],
                                o_sb[:, h, qc * P : (qc + 1) * P],
                                wo_sb[:, h, np2 * 2 + nn, :],
                                start=(h == 0), stop=(h == H_PER_CORE - 1),
                            )
                    ot = wosb.tile([P, Q], bf16, tag="wout", name=f"wt{idx}")
                    # evacuate the two 512-wide halves on DVE and ACT in
                    # parallel (Copy shares the Exp table: no reload)
                    nc.vector.tensor_copy(ot[:, 0:512], w_ps[:, 0:512])
                    nc.scalar.activation(ot[:, 512:1024], w_ps[:, 512:1024], AF.Copy)
                    eng = nc.sync if idx % 2 == 0 else nc.scalar
                    eng.dma_start(
                        out_d[qc * P : (qc + 1) * P,
                              np2 * 1024 : (np2 + 1) * 1024], ot[:]
                    )
            if debug:
                nc.sync.dma_start(dbg_qt_d[:], qt_sb[:].rearrange("p h q -> p (h q)"))
                nc.sync.dma_start(dbg_kt_d[:], kt_sb[:])
                nc.gpsimd.dma_start(dbg_v_d[:], v_sb[:].rearrange("p t d -> p (t d)"))
                nc.sync.dma_start(dbg_on_d[:], o_sb[:].rearrange("p h q -> p (h q)"))
                nc.sync.dma_start(dbg_wq_d[:], wq_sb[:].rearrange("p c d -> p (c d)"))
                nc.sync.dma_start(dbg_wkv_d[:], wkv_sb[:].rearrange("p c d -> p (c d)"))

    nc.finalize()
    return nc


def _host_prep(hidden_states, attention_mask, position_ids, past_k, past_v, Wq, Wk, Wv, Wo):
    """Build the 8 per-core input maps (numpy, bf16 compute layouts)."""
    import ml_dtypes

    bf16 = ml_dtypes.bfloat16
    f32 = np.float32
    hs = np.asarray(hidden_states, f32).reshape(Q, HID)
    xt = np.ascontiguousarray(hs.T).astype(bf16)                  # [HID, Q]
    pos = np.asarray(position_ids).reshape(Q).astype(np.float64)
    inv_freq = 1.0 / (THETA ** (np.arange(0, HD, 2, dtype=f32).astype(np.float64) / HD))
    freqs = inv_freq[:, None] * pos[None, :]                      # [64, Q]
    cos_t = np.concatenate([np.cos(freqs), np.cos(freqs)], 0).astype(f32)   # [128, Q]
    sinS_t = np.concatenate([-np.sin(freqs), np.sin(freqs)], 0).astype(f32)

    mask = np.asarray(attention_mask, f32)
    mask_nonzero = bool(np.any(mask))
    emask_t = None
    if mask_nonzero:
        emask_t = np.ascontiguousarray(np.exp(mask.reshape(Q, KV)).T).astype(bf16)

    Wq = np.asarray(Wq, f32); Wk = np.asarray(Wk, f32)
    Wv = np.asarray(Wv, f32); Wo = np.asarray(Wo, f32)
    past_k = np.asarray(past_k, f32); past_v = np.asarray(past_v, f32)

    in_maps = []
    for g in range(N_CORES):
        qrows = slice(g * DH, (g + 1) * DH)
        krows = slice(g * HD, (g + 1) * HD)
        m = {
            "xt": xt,
            "wqt": np.ascontiguousarray(Wq[qrows, :].T).astype(bf16),
            "wkvt": np.ascontiguousarray(
                np.concatenate([Wk[krows, :], Wv[krows, :]], axis=0).T
            ).astype(bf16),
            "wot": np.ascontiguousarray(Wo[:, qrows].T).astype(bf16),
            "past_kt": np.ascontiguousarray(past_k[0, g].T).astype(bf16),
            "past_v": np.ascontiguousarray(past_v[0, g]),
            "cos_t": cos_t,
            "sinS_t": sinS_t,
        }
        if mask_nonzero:
            m["expmask_t"] = emask_t
        in_maps.append(m)
    return in_maps, mask_nonzero


def kernel(hidden_states, attention_mask, position_ids, past_k, past_v, Wq, Wk, Wv, Wo,
           _trace=False):
    from concourse.bass_utils import run_bass_kernel_spmd

    in_maps, mask_nonzero = _host_prep(
        hidden_states, attention_mask, position_ids, past_k, past_v, Wq, Wk, Wv, Wo
    )
    key = ("k", mask_nonzero)
    if key not in _cache:
        _cache[key] = _build(mask_nonzero)
    nc = _cache[key]
    res = run_bass_kernel_spmd(nc, in_maps, core_ids=list(range(N_CORES)), trace=_trace)
    out = res.results[0]["out_partial"].astype(np.float64)
    for g in range(1, N_CORES):
        out += res.results[g]["out_partial"]
    kernel.last_exec_time_ns = res.exec_time_ns
    return out.astype(np.float32).reshape(B, Q, HID)
